# revision 17
# baseline (speedup 1.0000x reference)
"""Trainium2 Bass kernel for BDH recurrent (chunked linear) attention.

Problem shapes (hardcoded): Q_raw [2,16,2048,256] f32, V_raw [2,2048,1024] f32,
out [2,16,2048,1024] f32.  8 NeuronCores, data+head parallel: each core owns
4 (batch, head) pairs; V is shared across the 4 heads of a core's batch.

Math (reference semantics), per (b,h), chunks of 128:
  QR = rope(Q); KR = QR
  out_c = q_c @ state_{<c} + (q_c q_c^T  * strict_tril) v_c
  state += q_c^T v_c

RoPE is precomputed on the host (elementwise prep, like the bf16 cast and
layout permutes): the device receives the roped q in both the natural [t, n]
layout (the state update's lhsT) and the transposed [n, t] layout (the m1/G
lhsT), pair-deinterleaved into (evens | odds) planes.

Chunk size = superchunk = 128 (SUP=1): this minimizes total PE columns —
the state ops (q@state and q^T v) cost 2*T*N*D MACs regardless of chunking,
while the triangular G/PV part grows linearly with the superchunk width, so
per-chunk state casts buy the minimum matmul work.  The recurrent state
lives in PSUM (fp32, 4 banks) and is cast to a bf16 SBUF copy once per
chunk, split across the vector and scalar engines so the cast latency hides
under the chunk's G/PV matmuls.

Per-chunk emission order (PE): G(i) -> PV(i) -> m1(i) -> m4(i).  PV before
m1 lets the chunk's out PSUM banks evacuate early (during m1/m4), which is
what makes 3 out-PSUM banks enough; m4 last gives the next chunk's cast the
whole m1 stream to hide under.  All DRAM layouts are partition-major so
every DMA is 128 contiguous descriptors; the output is written
partition-major per chunk and un-permuted on host.
"""

import numpy as np
import ml_dtypes

import concourse.mybir as mybir
import concourse.tile as tile
from concourse import bacc
from concourse.bass import ds
from concourse.bass_utils import run_bass_kernel_spmd

B, NH, T, N, D = 2, 16, 2048, 256, 1024
P = 128          # partition / chunk size
NCH = T // P     # 16 chunks
SUP = 2          # chunks per superchunk
NSUP = NCH // SUP
NQR = NCH - SUP  # natural-layout q only feeds m4; last sup's m4 is skipped
HPC = 4          # (b,h) pairs per core
NCORES = 8
THETA = 2.0 ** 16
TWO_PI = 2.0 * np.pi

bf = mybir.dt.bfloat16
f32 = mybir.dt.float32
bf_np = ml_dtypes.bfloat16

mult = mybir.AluOpType.mult

# PSUM evacuation budget: every [128,512] fp32 PSUM read costs ~690ns on
# EITHER engine (PSUM fp32 source = 1x mode), so the per-superchunk work
# (4 state-cast quarters + 4 out-evac halves + 2 G masks) is split so each
# engine carries ~65%: v gets the h=0 cast quarters + masks + h=0 evacs,
# s gets the h=1 quarters + h=1 evacs.
OUT_EVAC_ENG = ("v", "s")


def _copy(nc, c, out, in_):
    if c == "s":
        nc.scalar.copy(out, in_)
    else:
        nc.vector.tensor_copy(out, in_)


def _emit_body(nc, tc, qn, qt, v, mskT, out):
    """Tile program for one core: 4 (b,h) pairs, full scan each."""
    with (
        tc.tile_pool(name="const", bufs=1) as constp,
        tc.tile_pool(name="qpool", bufs=2) as qpool,
        tc.tile_pool(name="gwork", bufs=6) as work,
        tc.tile_pool(name="outbuf", bufs=6) as outp,
        tc.tile_pool(name="statesb", bufs=2) as statep,
        tc.tile_pool(name="ps_state", bufs=1, space="PSUM") as ps_state,
        tc.tile_pool(name="ps_out", bufs=2, space="PSUM") as ps_out,
        tc.tile_pool(name="ps_g", bufs=2, space="PSUM") as ps_g,
    ):
        # resident constants.  v is split so the first chunks land first;
        # the mask is tiny and gates the first PV.
        msk_sb = constp.tile([P, 3 * P], bf)
        nc.sync.dma_start(msk_sb[:], mskT[:, :])
        v_sb = constp.tile([P, NCH, D], bf)
        nc.sync.dma_start(v_sb[:, :SUP], v[:, :SUP, :])
        nc.sync.dma_start(v_sb[:, SUP:8], v[:, SUP:8, :])
        nc.sync.dma_start(v_sb[:, 8:], v[:, 8:, :])

        for bh in range(HPC):
            qt_sb = qpool.tile([P, 2, T], bf, tag="qt")
            # pair 0 gates the startup ramp: land its first chunks first.
            qspans = [(0, 2 * P), (2 * P, T - 2 * P)] if bh == 0 else [(0, T)]
            for c0, w in qspans:
                nc.scalar.dma_start(qt_sb[:, 0, ds(c0, w)], qt[bh, 0, :, ds(c0, w)])
                nc.scalar.dma_start(qt_sb[:, 1, ds(c0, w)], qt[bh, 1, :, ds(c0, w)])
            qn_sb = qpool.tile([P, 2, NQR, P], bf, tag="qn")
            if bh == 0:
                nc.scalar.dma_start(qn_sb[:, :, :4], qn[bh, :, :, :4])
                nc.scalar.dma_start(qn_sb[:, :, 4:], qn[bh, :, :, 4:NQR])
            else:
                nc.scalar.dma_start(qn_sb[:], qn[bh, :, :, :NQR])

            state_ps = ps_state.tile([P, 2, D], f32, tag="state")

            # Both G blocks of a superchunk share one PSUM bank:
            # [:, 0:256] = G0 (diag(c0) | cross c0->c1), [:, 256:384] =
            # diag(c1).  The G matmuls for superchunk s+1 are emitted in
            # the middle of superchunk s (PE-prefetch), which widens the
            # window the state casts have before m1 reads them; the mask
            # TTs stay in superchunk s+1's own DVE-queue slot.
            def emit_G(s):
                g_ps = ps_g.tile([P, 3 * P], f32, tag="g", name="g_ps")
                j = s * SUP
                for m in range(2):
                    nc.tensor.matmul(
                        g_ps[:, ds(0, 2 * P)], qt_sb[:, m, ds(j * P, P)],
                        qt_sb[:, m, ds(j * P, 2 * P)],
                        start=(m == 0), stop=(m == 1),
                        skip_group_check=True,
                    )
                for m in range(2):
                    nc.tensor.matmul(
                        g_ps[:, ds(2 * P, P)], qt_sb[:, m, ds((j + 1) * P, P)],
                        qt_sb[:, m, ds((j + 1) * P, P)],
                        start=(m == 0), stop=(m == 1),
                        skip_group_check=True,
                    )
                return g_ps

            g_ps_cur = emit_G(0)
            for s in range(NSUP):
                # DVE-queue order within a superchunk is load-bearing:
                # mask-a (gates PV(c0)) first, then the two v-side cast
                # quarters, then mask-b (gates PV(c1)), then the out
                # evacuations.  The s-engine carries the other two cast
                # quarters and the h1 evacuations on its own queue.
                g_sb = work.tile([P, 3 * P], bf, tag="gsb", name="g_sb")
                nc.vector.tensor_tensor(
                    g_sb[:, ds(0, P)], g_ps_cur[:, ds(0, P)],
                    msk_sb[:, ds(0, P)], mult
                )
                # The two engines each write their OWN tile (cross-engine
                # writes to one tile get WAW-serialized by Tile, which
                # would chain all four quarters end-to-end): v holds
                # (m0,h0),(m1,h1); s holds (m0,h1),(m1,h0).
                stv = sts = None
                if s > 0:
                    stv = statep.tile([P, 2, 512], bf, tag="stv", name="stv")
                    sts = statep.tile([P, 2, 512], bf, tag="sts", name="sts")
                    nc.scalar.copy(sts[:, 0], state_ps[:, 0, ds(512, 512)])
                    nc.vector.tensor_copy(stv[:, 0], state_ps[:, 0, ds(0, 512)])
                    nc.scalar.copy(sts[:, 1], state_ps[:, 1, ds(0, 512)])
                    nc.vector.tensor_copy(stv[:, 1], state_ps[:, 1, ds(512, 512)])
                nc.vector.tensor_tensor(
                    g_sb[:, ds(P, 2 * P)], g_ps_cur[:, ds(P, 2 * P)],
                    msk_sb[:, ds(P, 2 * P)], mult
                )
                # m1 rhs lookup: (m, h) -> cast piece tile/slot
                st_piece = {
                    (0, 0): lambda: stv[:, 0], (0, 1): lambda: sts[:, 0],
                    (1, 0): lambda: sts[:, 1], (1, 1): lambda: stv[:, 1],
                }

                for ci in range(SUP):
                    i = s * SUP + ci
                    # out(i) = PV + m1, one PSUM accumulation group per h
                    # bank.  PV first: with m1/m4 streaming after, the
                    # chunk's banks close early enough that 2 out-PSUM
                    # bufs never stall the next chunk.
                    out_ps = [
                        ps_out.tile([P, 512], f32, tag="outp", name=f"out_ps{h}")
                        for h in range(2)
                    ]
                    first = True
                    for cj in range(ci + 1):
                        goff = (2 * cj + (ci - cj)) * P
                        for h in range(2):
                            nc.tensor.matmul(
                                out_ps[h][:],
                                g_sb[:, ds(goff, P)],
                                v_sb[:, s * SUP + cj, ds(h * 512, 512)],
                                start=first, stop=(s == 0 and cj == ci),
                                skip_group_check=True,
                            )
                        first = False

                    # PE-prefetch the next superchunk's G right here: it
                    # only needs qt, and it pushes m1 back by 320ns, which
                    # is the slack the state casts need.
                    if ci == 0 and s + 1 < NSUP:
                        g_ps_next = emit_G(s + 1)

                    if s > 0:
                        # m-outer / h-inner: consecutive matmuls share lhsT
                        for m in range(2):
                            for h in range(2):
                                nc.tensor.matmul(
                                    out_ps[h][:], qt_sb[:, m, ds(i * P, P)],
                                    st_piece[(m, h)](),
                                    start=False, stop=(m == 1),
                                    skip_group_check=True,
                                )

                    # state += qr_i^T v_i, emitted last in the chunk: each
                    # (m, h) bank closes at ci==SUP-1 so the next cast
                    # quarters start as soon as their own bank retires.
                    if s < NSUP - 1:
                        for m in range(2):
                            for h in range(2):
                                dsl = ds(h * 512, 512)
                                nc.tensor.matmul(
                                    state_ps[:, m, dsl],
                                    qn_sb[:, m, i, :],
                                    v_sb[:, i, dsl],
                                    start=(i == 0),
                                    stop=(ci == SUP - 1),
                                    skip_group_check=True,
                                )

                    # evacuate + write back this chunk immediately: each
                    # half has its OWN staging tile (same cross-engine WAW
                    # rule as the casts) and its own DMA, h1's triggered
                    # from the otherwise-idle gpsimd queue.
                    ob0 = outp.tile([P, 512], bf, tag="out0", name="out_sb0")
                    ob1 = outp.tile([P, 512], bf, tag="out1", name="out_sb1")
                    nc.vector.tensor_copy(ob0[:], out_ps[0][:])
                    nc.scalar.copy(ob1[:], out_ps[1][:])
                    nc.sync.dma_start(out[bh, :, i, ds(0, 512)], ob0[:])
                    nc.gpsimd.dma_start(out[bh, :, i, ds(512, 512)], ob1[:])

                if s + 1 < NSUP:
                    g_ps_cur = g_ps_next


_BUILT = {}


def _build():
    if "nc" in _BUILT:
        return _BUILT["nc"]
    nc = bacc.Bacc(
        "TRN2", target_bir_lowering=False, debug=False,
        enable_asserts=True, num_devices=NCORES,
    )
    qn = nc.dram_tensor("qn", [HPC, P, 2, NCH, P], bf, kind="ExternalInput")
    qt = nc.dram_tensor("qt", [HPC, 2, P, T], bf, kind="ExternalInput")
    v = nc.dram_tensor("v", [P, NCH, D], bf, kind="ExternalInput")
    mskT = nc.dram_tensor("mskT", [P, 3 * P], bf, kind="ExternalInput")
    out = nc.dram_tensor("out", [HPC, P, NCH, D], bf, kind="ExternalOutput")
    with tile.TileContext(nc) as tc:
        _emit_body(nc, tc, qn, qt, v, mskT, out)
    nc.compile()
    _BUILT["nc"] = nc
    return nc


def _host_prep(Q_raw, V_raw):
    """Shard + precompute device inputs (bf16, partition-major layouts).

    RoPE is applied here in fp32 (matching reference._get_freqs/_rope
    exactly), then cast to bf16.
    """
    Q = np.asarray(Q_raw, dtype=np.float32)
    V = np.asarray(V_raw, dtype=np.float32)

    t = np.arange(N, dtype=np.float32)
    qq = np.floor(t / 2.0) * 2.0
    freqs = (1.0 / (THETA ** (qq / np.float32(N))) / np.float32(TWO_PI)).astype(
        np.float32
    )
    phases = np.arange(T, dtype=np.float32)[:, None] * freqs[None, :]
    ph = ((phases % 1.0) * np.float32(TWO_PI)).astype(np.float32)
    cosf = np.cos(ph).astype(np.float32)   # [T, N]; equal within (even, odd)
    sinf = np.sin(ph).astype(np.float32)
    even, odd = Q[..., 0::2], Q[..., 1::2]
    ce, se = cosf[:, 0::2], sinf[:, 0::2]  # [T, 128]
    qr_e = even * ce - odd * se            # [B, NH, T, 128]
    qr_o = odd * ce + even * se
    QRd = np.stack([qr_e, qr_o], axis=2).astype(bf_np)  # [B, NH, 2, T, 128]

    # natural layout  [b,h][p, half, c, k] = QRd[b, h, half, c*128+p, k]
    Qn = np.ascontiguousarray(
        QRd.reshape(B, NH, 2, NCH, P, P).transpose(0, 1, 4, 2, 3, 5)
    )  # [B, NH, P, 2, NCH, P]
    # transposed layout [b,h][half, k, t] = QRd[b, h, half, t, k]
    Qt = np.ascontiguousarray(QRd.transpose(0, 1, 2, 4, 3))  # [B, NH, 2, 128, T]

    # [strict-triu | ones | strict-triu]: diag(c0) mask, cross block
    # passthrough, diag(c1) mask — matches the shared-bank G layout.
    mskT = np.ones((P, 3 * P), np.float32)
    tri = np.triu(np.ones((P, P), np.float32), k=1)
    mskT[:, :P] = tri
    mskT[:, 2 * P:] = tri
    mskT = mskT.astype(bf_np)

    V16 = V.astype(bf_np)
    # v layout [P, NCH, D]: (p, c, d) = V[c*128+p, d]
    Vp = np.ascontiguousarray(V16.reshape(B, NCH, P, D).transpose(0, 2, 1, 3))

    in_maps = []
    for core in range(NCORES):
        b = core // (NCORES // B)
        hs = (core % (NCORES // B)) * HPC
        in_maps.append(
            {
                "qn": np.ascontiguousarray(Qn[b, hs : hs + HPC]),
                "qt": np.ascontiguousarray(Qt[b, hs : hs + HPC]),
                "v": Vp[b],
                "mskT": mskT,
            }
        )
    return in_maps


def _run(inputs, trace=False, **kw):
    nc = _build()
    in_maps = _host_prep(inputs["Q_raw"], inputs["V_raw"])
    res = run_bass_kernel_spmd(nc, in_maps, list(range(NCORES)), trace=trace, **kw)
    out = np.empty((B, NH, T, D), dtype=np.float32)
    for core in range(NCORES):
        b = core // (NCORES // B)
        hs = (core % (NCORES // B)) * HPC
        # device out: [HPC, P, NCH, D] partition-major -> [HPC, T, D]
        o = res.results[core]["out"].astype(np.float32)
        out[b, hs : hs + HPC] = o.transpose(0, 2, 1, 3).reshape(HPC, T, D)
    return out, res


def kernel(**inputs):
    out, _ = _run(inputs)
    return out


# revision 20
# speedup vs baseline: 1.2643x; 1.2643x over previous
"""Trainium2 Bass kernel for BDH recurrent (chunked linear) attention.

Problem shapes (hardcoded): Q_raw [2,16,2048,256] f32, V_raw [2,2048,1024] f32,
out [2,16,2048,1024] f32.  8 NeuronCores, data+head parallel: each core owns
4 (batch, head) pairs; V is shared across the 4 heads of a core's batch.

Math (reference semantics), per (b,h), chunks of 128:
  QR = rope(Q); KR = QR
  out_c = q_c @ state_{<c} + (q_c q_c^T  * strict_tril) v_c
  state += q_c^T v_c

RoPE is precomputed on the host (elementwise prep, like the bf16 cast and
layout permutes): the device receives the roped q in both the natural [t, n]
layout (the state update's lhsT) and the transposed [n, t] layout (the m1/G
lhsT), pair-deinterleaved into (evens | odds) planes.

Chunk size = superchunk = 128 (SUP=1): this minimizes total PE columns —
the state ops (q@state and q^T v) cost 2*T*N*D MACs regardless of chunking,
while the triangular G/PV part grows linearly with the superchunk width, so
per-chunk state casts buy the minimum matmul work.  The recurrent state
lives in PSUM (fp32, 4 banks) and is cast to a bf16 SBUF copy once per
chunk, split across the vector and scalar engines so the cast latency hides
under the chunk's G/PV matmuls.

Per-chunk emission order (PE): G(i) -> PV(i) -> m1(i) -> m4(i).  PV before
m1 lets the chunk's out PSUM banks evacuate early (during m1/m4), which is
what makes 3 out-PSUM banks enough; m4 last gives the next chunk's cast the
whole m1 stream to hide under.  All DRAM layouts are partition-major so
every DMA is 128 contiguous descriptors; the output is written
partition-major per chunk and un-permuted on host.
"""

import numpy as np
import ml_dtypes

import concourse.mybir as mybir
import concourse.tile as tile
from concourse import bacc
from concourse.bass import ds
from concourse.bass_utils import run_bass_kernel_spmd

B, NH, T, N, D = 2, 16, 2048, 256, 1024
P = 128          # partition / chunk size
NCH = T // P     # 16 chunks
SUP = 2          # chunks per superchunk
NSUP = NCH // SUP
NQR = NCH - SUP  # natural-layout q only feeds m4; last sup's m4 is skipped
HPC = 4          # (b,h) pairs per core
NCORES = 8
THETA = 2.0 ** 16
TWO_PI = 2.0 * np.pi

bf = mybir.dt.bfloat16
f32 = mybir.dt.float32
bf_np = ml_dtypes.bfloat16

mult = mybir.AluOpType.mult

# PSUM evacuation budget: every [128,512] fp32 PSUM read costs ~690ns on
# EITHER engine (PSUM fp32 source = 1x mode), so the per-superchunk work
# (4 state-cast quarters + 4 out-evac halves + 2 G masks) is split so each
# engine carries ~65%: v gets the h=0 cast quarters + masks + h=0 evacs,
# s gets the h=1 quarters + h=1 evacs.
OUT_EVAC_ENG = ("v", "s")


def _copy(nc, c, out, in_):
    if c == "s":
        nc.scalar.copy(out, in_)
    else:
        nc.vector.tensor_copy(out, in_)


def _emit_body(nc, tc, qn, qt, v, mskT, out):
    """Tile program for one core: 4 (b,h) pairs, full scan each."""
    with (
        tc.tile_pool(name="const", bufs=1) as constp,
        tc.tile_pool(name="qpool", bufs=2) as qpool,
        tc.tile_pool(name="gwork", bufs=6) as work,
        tc.tile_pool(name="outbuf", bufs=6) as outp,
        tc.tile_pool(name="statesb", bufs=2) as statep,
        tc.tile_pool(name="ps_state", bufs=1, space="PSUM") as ps_state,
        tc.tile_pool(name="ps_out", bufs=2, space="PSUM") as ps_out,
        tc.tile_pool(name="ps_g", bufs=2, space="PSUM") as ps_g,
    ):
        # resident constants.  v is split so the first chunks land first;
        # the mask is tiny and gates the first PV.
        msk_sb = constp.tile([P, 3 * P], bf)
        nc.sync.dma_start(msk_sb[:], mskT[:, :])
        v_sb = constp.tile([P, NCH, D], bf)
        nc.sync.dma_start(v_sb[:, :SUP], v[:, :SUP, :])
        nc.sync.dma_start(v_sb[:, SUP:8], v[:, SUP:8, :])
        nc.sync.dma_start(v_sb[:, 8:], v[:, 8:, :])

        for bh in range(HPC):
            qt_sb = qpool.tile([P, 2, T], bf, tag="qt")
            # pair 0 gates the startup ramp: land its first chunks first.
            qspans = [(0, 2 * P), (2 * P, T - 2 * P)] if bh == 0 else [(0, T)]
            for c0, w in qspans:
                nc.scalar.dma_start(qt_sb[:, 0, ds(c0, w)], qt[bh, 0, :, ds(c0, w)])
                nc.scalar.dma_start(qt_sb[:, 1, ds(c0, w)], qt[bh, 1, :, ds(c0, w)])
            qn_sb = qpool.tile([P, 2, NQR, P], bf, tag="qn")
            if bh == 0:
                nc.scalar.dma_start(qn_sb[:, :, :4], qn[bh, :, :, :4])
                nc.scalar.dma_start(qn_sb[:, :, 4:], qn[bh, :, :, 4:NQR])
            else:
                nc.scalar.dma_start(qn_sb[:], qn[bh, :, :, :NQR])

            # The state accumulator is TWO PSUM tensors, one per cast
            # engine (v reads stv_ps, s reads sts_ps): PSUM accesses to a
            # single tensor are serialized by Tile even across engines, so
            # a single 4-bank state tensor would chain the four cast
            # quarters end-to-end.  stv holds (m0,h0),(m1,h1); sts holds
            # (m0,h1),(m1,h0).
            stv_ps = ps_state.tile([P, 2, 512], f32, tag="state_v")
            sts_ps = ps_state.tile([P, 2, 512], f32, tag="state_s")
            # m4 target lookup: (m, h) -> PSUM slot
            m4_dst = {
                (0, 0): stv_ps[:, 0], (1, 1): stv_ps[:, 1],
                (0, 1): sts_ps[:, 0], (1, 0): sts_ps[:, 1],
            }

            # Both G blocks of a superchunk share one PSUM bank:
            # [:, 0:256] = G0 (diag(c0) | cross c0->c1), [:, 256:384] =
            # diag(c1).  The G matmuls for superchunk s+1 are emitted in
            # the middle of superchunk s (PE-prefetch), which widens the
            # window the state casts have before m1 reads them; the mask
            # TTs stay in superchunk s+1's own DVE-queue slot.
            def emit_G(s):
                g_ps = ps_g.tile([P, 3 * P], f32, tag="g", name="g_ps")
                j = s * SUP
                for m in range(2):
                    nc.tensor.matmul(
                        g_ps[:, ds(0, 2 * P)], qt_sb[:, m, ds(j * P, P)],
                        qt_sb[:, m, ds(j * P, 2 * P)],
                        start=(m == 0), stop=(m == 1),
                        skip_group_check=True,
                    )
                for m in range(2):
                    nc.tensor.matmul(
                        g_ps[:, ds(2 * P, P)], qt_sb[:, m, ds((j + 1) * P, P)],
                        qt_sb[:, m, ds((j + 1) * P, P)],
                        start=(m == 0), stop=(m == 1),
                        skip_group_check=True,
                    )
                return g_ps

            g_ps_cur = emit_G(0)
            for s in range(NSUP):
                # DVE-queue order within a superchunk is load-bearing:
                # mask-a (gates PV(c0)) first, then the two v-side cast
                # quarters, then mask-b (gates PV(c1)), then the out
                # evacuations.  The s-engine carries the other two cast
                # quarters and the h1 evacuations on its own queue.
                g_sb = work.tile([P, 3 * P], bf, tag="gsb", name="g_sb")
                nc.vector.tensor_tensor(
                    g_sb[:, ds(0, P)], g_ps_cur[:, ds(0, P)],
                    msk_sb[:, ds(0, P)], mult
                )
                # The two engines each write their OWN tile (cross-engine
                # writes to one tile get WAW-serialized by Tile, which
                # would chain all four quarters end-to-end): v holds
                # (m0,h0),(m1,h1); s holds (m0,h1),(m1,h0).
                stv = sts = None
                if s > 0:
                    stv = statep.tile([P, 2, 512], bf, tag="stv", name="stv")
                    sts = statep.tile([P, 2, 512], bf, tag="sts", name="sts")
                    nc.scalar.copy(sts[:, 0], sts_ps[:, 0])
                    nc.vector.tensor_copy(stv[:, 0], stv_ps[:, 0])
                    nc.scalar.copy(sts[:, 1], sts_ps[:, 1])
                    nc.vector.tensor_copy(stv[:, 1], stv_ps[:, 1])
                nc.vector.tensor_tensor(
                    g_sb[:, ds(P, 2 * P)], g_ps_cur[:, ds(P, 2 * P)],
                    msk_sb[:, ds(P, 2 * P)], mult
                )
                # m1 rhs lookup: (m, h) -> cast piece tile/slot
                st_piece = {
                    (0, 0): lambda: stv[:, 0], (0, 1): lambda: sts[:, 0],
                    (1, 0): lambda: sts[:, 1], (1, 1): lambda: stv[:, 1],
                }

                for ci in range(SUP):
                    i = s * SUP + ci
                    # out(i) = PV + m1, one PSUM accumulation group per h
                    # bank.  PV first: with m1/m4 streaming after, the
                    # chunk's banks close early enough that 2 out-PSUM
                    # bufs never stall the next chunk.
                    out_ps = [
                        ps_out.tile([P, 512], f32, tag="outp", name=f"out_ps{h}")
                        for h in range(2)
                    ]
                    first = True
                    for cj in range(ci + 1):
                        goff = (2 * cj + (ci - cj)) * P
                        for h in range(2):
                            nc.tensor.matmul(
                                out_ps[h][:],
                                g_sb[:, ds(goff, P)],
                                v_sb[:, s * SUP + cj, ds(h * 512, 512)],
                                start=first, stop=(s == 0 and cj == ci),
                                skip_group_check=True,
                            )
                        first = False

                    # PE-prefetch the next superchunk's G right here: it
                    # only needs qt, and it pushes m1 back by 320ns, which
                    # is the slack the state casts need.
                    if ci == 0 and s + 1 < NSUP:
                        g_ps_next = emit_G(s + 1)

                    if s > 0:
                        # m-outer / h-inner: consecutive matmuls share lhsT
                        for m in range(2):
                            for h in range(2):
                                nc.tensor.matmul(
                                    out_ps[h][:], qt_sb[:, m, ds(i * P, P)],
                                    st_piece[(m, h)](),
                                    start=False, stop=(m == 1),
                                    skip_group_check=True,
                                )

                    # state += qr_i^T v_i, emitted last in the chunk.  The
                    # s-tensor quarters go first so the s engine's casts
                    # (which feed m1 matmuls 2 and 3) start earliest.
                    if s < NSUP - 1:
                        for m, h in ((0, 1), (1, 0), (0, 0), (1, 1)):
                            nc.tensor.matmul(
                                m4_dst[(m, h)],
                                qn_sb[:, m, i, :],
                                v_sb[:, i, ds(h * 512, 512)],
                                start=(i == 0),
                                stop=(ci == SUP - 1),
                                skip_group_check=True,
                            )

                    # evacuate + write back this chunk immediately: each
                    # half has its OWN staging tile (same cross-engine WAW
                    # rule as the casts) and its own DMA, h1's triggered
                    # from the otherwise-idle gpsimd queue.
                    ob0 = outp.tile([P, 512], bf, tag="out0", name="out_sb0")
                    ob1 = outp.tile([P, 512], bf, tag="out1", name="out_sb1")
                    nc.vector.tensor_copy(ob0[:], out_ps[0][:])
                    nc.scalar.copy(ob1[:], out_ps[1][:])
                    nc.sync.dma_start(out[bh, :, i, ds(0, 512)], ob0[:])
                    nc.gpsimd.dma_start(out[bh, :, i, ds(512, 512)], ob1[:])

                if s + 1 < NSUP:
                    g_ps_cur = g_ps_next


_BUILT = {}


def _build():
    if "nc" in _BUILT:
        return _BUILT["nc"]
    nc = bacc.Bacc(
        "TRN2", target_bir_lowering=False, debug=False,
        enable_asserts=True, num_devices=NCORES,
    )
    qn = nc.dram_tensor("qn", [HPC, P, 2, NCH, P], bf, kind="ExternalInput")
    qt = nc.dram_tensor("qt", [HPC, 2, P, T], bf, kind="ExternalInput")
    v = nc.dram_tensor("v", [P, NCH, D], bf, kind="ExternalInput")
    mskT = nc.dram_tensor("mskT", [P, 3 * P], bf, kind="ExternalInput")
    out = nc.dram_tensor("out", [HPC, P, NCH, D], bf, kind="ExternalOutput")
    with tile.TileContext(nc) as tc:
        _emit_body(nc, tc, qn, qt, v, mskT, out)
    nc.compile()
    _BUILT["nc"] = nc
    return nc


def _host_prep(Q_raw, V_raw):
    """Shard + precompute device inputs (bf16, partition-major layouts).

    RoPE is applied here in fp32 (matching reference._get_freqs/_rope
    exactly), then cast to bf16.
    """
    Q = np.asarray(Q_raw, dtype=np.float32)
    V = np.asarray(V_raw, dtype=np.float32)

    t = np.arange(N, dtype=np.float32)
    qq = np.floor(t / 2.0) * 2.0
    freqs = (1.0 / (THETA ** (qq / np.float32(N))) / np.float32(TWO_PI)).astype(
        np.float32
    )
    phases = np.arange(T, dtype=np.float32)[:, None] * freqs[None, :]
    ph = ((phases % 1.0) * np.float32(TWO_PI)).astype(np.float32)
    cosf = np.cos(ph).astype(np.float32)   # [T, N]; equal within (even, odd)
    sinf = np.sin(ph).astype(np.float32)
    even, odd = Q[..., 0::2], Q[..., 1::2]
    ce, se = cosf[:, 0::2], sinf[:, 0::2]  # [T, 128]
    qr_e = even * ce - odd * se            # [B, NH, T, 128]
    qr_o = odd * ce + even * se
    QRd = np.stack([qr_e, qr_o], axis=2).astype(bf_np)  # [B, NH, 2, T, 128]

    # natural layout  [b,h][p, half, c, k] = QRd[b, h, half, c*128+p, k]
    Qn = np.ascontiguousarray(
        QRd.reshape(B, NH, 2, NCH, P, P).transpose(0, 1, 4, 2, 3, 5)
    )  # [B, NH, P, 2, NCH, P]
    # transposed layout [b,h][half, k, t] = QRd[b, h, half, t, k]
    Qt = np.ascontiguousarray(QRd.transpose(0, 1, 2, 4, 3))  # [B, NH, 2, 128, T]

    # [strict-triu | ones | strict-triu]: diag(c0) mask, cross block
    # passthrough, diag(c1) mask — matches the shared-bank G layout.
    mskT = np.ones((P, 3 * P), np.float32)
    tri = np.triu(np.ones((P, P), np.float32), k=1)
    mskT[:, :P] = tri
    mskT[:, 2 * P:] = tri
    mskT = mskT.astype(bf_np)

    V16 = V.astype(bf_np)
    # v layout [P, NCH, D]: (p, c, d) = V[c*128+p, d]
    Vp = np.ascontiguousarray(V16.reshape(B, NCH, P, D).transpose(0, 2, 1, 3))

    in_maps = []
    for core in range(NCORES):
        b = core // (NCORES // B)
        hs = (core % (NCORES // B)) * HPC
        in_maps.append(
            {
                "qn": np.ascontiguousarray(Qn[b, hs : hs + HPC]),
                "qt": np.ascontiguousarray(Qt[b, hs : hs + HPC]),
                "v": Vp[b],
                "mskT": mskT,
            }
        )
    return in_maps


def _run(inputs, trace=False, **kw):
    nc = _build()
    in_maps = _host_prep(inputs["Q_raw"], inputs["V_raw"])
    res = run_bass_kernel_spmd(nc, in_maps, list(range(NCORES)), trace=trace, **kw)
    out = np.empty((B, NH, T, D), dtype=np.float32)
    for core in range(NCORES):
        b = core // (NCORES // B)
        hs = (core % (NCORES // B)) * HPC
        # device out: [HPC, P, NCH, D] partition-major -> [HPC, T, D]
        o = res.results[core]["out"].astype(np.float32)
        out[b, hs : hs + HPC] = o.transpose(0, 2, 1, 3).reshape(HPC, T, D)
    return out, res


def kernel(**inputs):
    out, _ = _run(inputs)
    return out


# revision 25
# speedup vs baseline: 1.3547x; 1.0714x over previous
"""Trainium2 Bass kernel for BDH recurrent (chunked linear) attention.

Problem shapes (hardcoded): Q_raw [2,16,2048,256] f32, V_raw [2,2048,1024] f32,
out [2,16,2048,1024] f32.  8 NeuronCores, data+head parallel: each core owns
4 (batch, head) pairs; V is shared across the 4 heads of a core's batch.

Math (reference semantics), per (b,h), chunks of 128:
  QR = rope(Q); KR = QR
  out_c = q_c @ state_{<c} + (q_c q_c^T  * strict_tril) v_c
  state += q_c^T v_c

RoPE is precomputed on the host (elementwise prep, like the bf16 cast and
layout permutes): the device receives the roped q in both the natural [t, n]
layout (the state update's lhsT) and the transposed [n, t] layout (the m1/G
lhsT), pair-deinterleaved into (evens | odds) planes.

Chunk size = superchunk = 128 (SUP=1): this minimizes total PE columns —
the state ops (q@state and q^T v) cost 2*T*N*D MACs regardless of chunking,
while the triangular G/PV part grows linearly with the superchunk width, so
per-chunk state casts buy the minimum matmul work.  The recurrent state
lives in PSUM (fp32, 4 banks) and is cast to a bf16 SBUF copy once per
chunk, split across the vector and scalar engines so the cast latency hides
under the chunk's G/PV matmuls.

Per-chunk emission order (PE): G(i) -> PV(i) -> m1(i) -> m4(i).  PV before
m1 lets the chunk's out PSUM banks evacuate early (during m1/m4), which is
what makes 3 out-PSUM banks enough; m4 last gives the next chunk's cast the
whole m1 stream to hide under.  All DRAM layouts are partition-major so
every DMA is 128 contiguous descriptors; the output is written
partition-major per chunk and un-permuted on host.
"""

import numpy as np
import ml_dtypes

import concourse.mybir as mybir
import concourse.tile as tile
from concourse import bacc
from concourse.bass import ds
from concourse.bass_utils import run_bass_kernel_spmd

B, NH, T, N, D = 2, 16, 2048, 256, 1024
P = 128          # partition / chunk size
NCH = T // P     # 16 chunks
SUP = 2          # chunks per superchunk
NSUP = NCH // SUP
NQR = NCH - SUP  # natural-layout q only feeds m4; last sup's m4 is skipped
HPC = 4          # (b,h) pairs per core
NCORES = 8
THETA = 2.0 ** 16
TWO_PI = 2.0 * np.pi

bf = mybir.dt.bfloat16
f32 = mybir.dt.float32
bf_np = ml_dtypes.bfloat16

mult = mybir.AluOpType.mult

# PSUM evacuation budget: every [128,512] fp32 PSUM read costs ~690ns on
# EITHER engine (PSUM fp32 source = 1x mode), so the per-superchunk work
# (4 state-cast quarters + 4 out-evac halves + 2 G masks) is split so each
# engine carries ~65%: v gets the h=0 cast quarters + masks + h=0 evacs,
# s gets the h=1 quarters + h=1 evacs.
OUT_EVAC_ENG = ("v", "s")


def _copy(nc, c, out, in_):
    if c == "s":
        nc.scalar.copy(out, in_)
    else:
        nc.vector.tensor_copy(out, in_)


def _emit_body(nc, tc, qn, qt, v, mskT, out):
    """Tile program for one core: 4 (b,h) pairs, full scan each."""
    with (
        tc.tile_pool(name="const", bufs=1) as constp,
        tc.tile_pool(name="qpool", bufs=2) as qpool,
        tc.tile_pool(name="gwork", bufs=6) as work,
        tc.tile_pool(name="outbuf", bufs=6) as outp,
        tc.tile_pool(name="statesb", bufs=2) as statep,
        tc.tile_pool(name="ps_state", bufs=1, space="PSUM") as ps_state,
        tc.tile_pool(name="ps_out", bufs=2, space="PSUM") as ps_out,
        tc.tile_pool(name="ps_g", bufs=2, space="PSUM") as ps_g,
    ):
        # resident constants.  The mask is tiny and gates the first PV; v's
        # first superchunk gates the first PV too.  Everything pair 0
        # doesn't need IMMEDIATELY is deferred (emitted inside the
        # superchunk loop) so the startup DMA bandwidth goes to the
        # critical slices.
        msk_sb = constp.tile([P, 3 * P], bf)
        nc.sync.dma_start(msk_sb[:], mskT[:, :])
        v_sb = constp.tile([P, NCH, D], bf)
        nc.sync.dma_start(v_sb[:, :SUP], v[:, :SUP, :])

        for bh in range(HPC):
            qt_sb = qpool.tile([P, 2, T], bf, tag="qt")
            qn_sb = qpool.tile([P, NQR, 2, P], bf, tag="qn")
            if bh == 0:
                # staggered: each slice lands just ahead of the superchunk
                # that needs it, instead of flooding the rings up front.
                for c0, w in [(0, 2 * P), (2 * P, 6 * P), (8 * P, T - 8 * P)]:
                    nc.scalar.dma_start(
                        qt_sb[:, 0, ds(c0, w)], qt[bh, 0, :, ds(c0, w)]
                    )
                    nc.scalar.dma_start(
                        qt_sb[:, 1, ds(c0, w)], qt[bh, 1, :, ds(c0, w)]
                    )
                    k0, k1 = c0 // P, min((c0 + w) // P, NQR)
                    nc.scalar.dma_start(
                        qn_sb[:, k0:k1], qn[bh, :, k0:k1]
                    )
                nc.sync.dma_start(v_sb[:, SUP:4], v[:, SUP:4, :])
                nc.sync.dma_start(v_sb[:, 4:8], v[:, 4:8, :])
                nc.sync.dma_start(v_sb[:, 8:], v[:, 8:, :])
            else:
                nc.scalar.dma_start(qt_sb[:, 0], qt[bh, 0])
                nc.scalar.dma_start(qt_sb[:, 1], qt[bh, 1])
                nc.scalar.dma_start(qn_sb[:], qn[bh, :, :NQR])

            # The state accumulator is TWO PSUM tensors, one per cast
            # engine (v reads stv_ps, s reads sts_ps): PSUM accesses to a
            # single tensor are serialized by Tile even across engines, so
            # a single 4-bank state tensor would chain the four cast
            # quarters end-to-end.  stv holds (m0,h0),(m1,h1); sts holds
            # (m0,h1),(m1,h0).
            stv_ps = ps_state.tile([P, 2, 512], f32, tag="state_v")
            sts_ps = ps_state.tile([P, 2, 512], f32, tag="state_s")
            # m4 target lookup: (m, h) -> PSUM slot
            m4_dst = {
                (0, 0): stv_ps[:, 0], (1, 1): stv_ps[:, 1],
                (0, 1): sts_ps[:, 0], (1, 0): sts_ps[:, 1],
            }

            # Both G blocks of a superchunk share one PSUM bank:
            # [:, 0:256] = G0 (diag(c0) | cross c0->c1), [:, 256:384] =
            # diag(c1).  The G matmuls for superchunk s+1 are emitted in
            # the middle of superchunk s (PE-prefetch), which widens the
            # window the state casts have before m1 reads them; the mask
            # TTs stay in superchunk s+1's own DVE-queue slot.
            def emit_G(s):
                g_ps = ps_g.tile([P, 3 * P], f32, tag="g", name="g_ps")
                j = s * SUP
                for m in range(2):
                    nc.tensor.matmul(
                        g_ps[:, ds(0, 2 * P)], qt_sb[:, m, ds(j * P, P)],
                        qt_sb[:, m, ds(j * P, 2 * P)],
                        start=(m == 0), stop=(m == 1),
                        skip_group_check=True,
                    )
                for m in range(2):
                    nc.tensor.matmul(
                        g_ps[:, ds(2 * P, P)], qt_sb[:, m, ds((j + 1) * P, P)],
                        qt_sb[:, m, ds((j + 1) * P, P)],
                        start=(m == 0), stop=(m == 1),
                        skip_group_check=True,
                    )
                return g_ps

            g_ps_cur = emit_G(0)
            for s in range(NSUP):
                # DVE-queue order within a superchunk is load-bearing:
                # mask-a (gates PV(c0)) first, then the two v-side cast
                # quarters, then mask-b (gates PV(c1)), then the out
                # evacuations.  The s-engine carries the other two cast
                # quarters and the h1 evacuations on its own queue.
                g_sb = work.tile([P, 3 * P], bf, tag="gsb", name="g_sb")
                nc.vector.tensor_tensor(
                    g_sb[:, ds(0, P)], g_ps_cur[:, ds(0, P)],
                    msk_sb[:, ds(0, P)], mult
                )
                # The two engines each write their OWN tile (cross-engine
                # writes to one tile get WAW-serialized by Tile, which
                # would chain all four quarters end-to-end): v holds
                # (m0,h0),(m1,h1); s holds (m0,h1),(m1,h0).
                stv = sts = None
                if s > 0:
                    stv = statep.tile([P, 2, 512], bf, tag="stv", name="stv")
                    sts = statep.tile([P, 2, 512], bf, tag="sts", name="sts")
                    nc.scalar.copy(sts[:, 0], sts_ps[:, 0])
                    nc.vector.tensor_copy(stv[:, 0], stv_ps[:, 0])
                    nc.scalar.copy(sts[:, 1], sts_ps[:, 1])
                    nc.vector.tensor_copy(stv[:, 1], stv_ps[:, 1])
                nc.vector.tensor_tensor(
                    g_sb[:, ds(P, 2 * P)], g_ps_cur[:, ds(P, 2 * P)],
                    msk_sb[:, ds(P, 2 * P)], mult
                )
                # m1 rhs lookup: (m, h) -> cast piece tile/slot
                st_piece = {
                    (0, 0): lambda: stv[:, 0], (0, 1): lambda: sts[:, 0],
                    (1, 0): lambda: sts[:, 1], (1, 1): lambda: stv[:, 1],
                }

                for ci in range(SUP):
                    i = s * SUP + ci
                    # out(i) = PV + m1, one PSUM accumulation group per h
                    # bank.  PV first: with m1/m4 streaming after, the
                    # chunk's banks close early enough that 2 out-PSUM
                    # bufs never stall the next chunk.
                    out_ps = [
                        ps_out.tile([P, 512], f32, tag="outp", name=f"out_ps{h}")
                        for h in range(2)
                    ]
                    first = True
                    for cj in range(ci + 1):
                        goff = (2 * cj + (ci - cj)) * P
                        for h in range(2):
                            nc.tensor.matmul(
                                out_ps[h][:],
                                g_sb[:, ds(goff, P)],
                                v_sb[:, s * SUP + cj, ds(h * 512, 512)],
                                start=first, stop=(s == 0 and cj == ci),
                                skip_group_check=True,
                            )
                        first = False

                    # PE-prefetch the next superchunk's G right here: it
                    # only needs qt, and it pushes m1 back by 320ns, which
                    # is the slack the state casts need.
                    if ci == 0 and s + 1 < NSUP:
                        g_ps_next = emit_G(s + 1)

                    if s > 0:
                        # m-outer / h-inner: consecutive matmuls share lhsT
                        for m in range(2):
                            for h in range(2):
                                nc.tensor.matmul(
                                    out_ps[h][:], qt_sb[:, m, ds(i * P, P)],
                                    st_piece[(m, h)](),
                                    start=False, stop=(m == 1),
                                    skip_group_check=True,
                                )

                    # state += qr_i^T v_i, emitted last in the chunk.  The
                    # s-tensor quarters go first so the s engine's casts
                    # (which feed m1 matmuls 2 and 3) start earliest.
                    if s < NSUP - 1:
                        for m, h in ((0, 1), (1, 0), (0, 0), (1, 1)):
                            nc.tensor.matmul(
                                m4_dst[(m, h)],
                                qn_sb[:, i, m, :],
                                v_sb[:, i, ds(h * 512, 512)],
                                start=(i == 0),
                                stop=(ci == SUP - 1),
                                skip_group_check=True,
                            )

                    # evacuate + write back this chunk immediately: each
                    # half has its OWN staging tile (same cross-engine WAW
                    # rule as the casts) and its own DMA, h1's triggered
                    # from the otherwise-idle gpsimd queue.
                    ob0 = outp.tile([P, 512], bf, tag="out0", name="out_sb0")
                    ob1 = outp.tile([P, 512], bf, tag="out1", name="out_sb1")
                    nc.vector.tensor_copy(ob0[:], out_ps[0][:])
                    nc.scalar.copy(ob1[:], out_ps[1][:])
                    nc.sync.dma_start(out[bh, :, i, ds(0, 512)], ob0[:])
                    nc.sync.dma_start(out[bh, :, i, ds(512, 512)], ob1[:])

                if s + 1 < NSUP:
                    g_ps_cur = g_ps_next


_BUILT = {}


def _build():
    if "nc" in _BUILT:
        return _BUILT["nc"]
    nc = bacc.Bacc(
        "TRN2", target_bir_lowering=False, debug=False,
        enable_asserts=True, num_devices=NCORES,
    )
    qn = nc.dram_tensor("qn", [HPC, P, NCH, 2, P], bf, kind="ExternalInput")
    qt = nc.dram_tensor("qt", [HPC, 2, P, T], bf, kind="ExternalInput")
    v = nc.dram_tensor("v", [P, NCH, D], bf, kind="ExternalInput")
    mskT = nc.dram_tensor("mskT", [P, 3 * P], bf, kind="ExternalInput")
    out = nc.dram_tensor("out", [HPC, P, NCH, D], bf, kind="ExternalOutput")
    with tile.TileContext(nc) as tc:
        _emit_body(nc, tc, qn, qt, v, mskT, out)
    nc.compile()
    _BUILT["nc"] = nc
    return nc


def _host_prep(Q_raw, V_raw):
    """Shard + precompute device inputs (bf16, partition-major layouts).

    RoPE is applied here in fp32 (matching reference._get_freqs/_rope
    exactly), then cast to bf16.
    """
    Q = np.asarray(Q_raw, dtype=np.float32)
    V = np.asarray(V_raw, dtype=np.float32)

    t = np.arange(N, dtype=np.float32)
    qq = np.floor(t / 2.0) * 2.0
    freqs = (1.0 / (THETA ** (qq / np.float32(N))) / np.float32(TWO_PI)).astype(
        np.float32
    )
    phases = np.arange(T, dtype=np.float32)[:, None] * freqs[None, :]
    ph = ((phases % 1.0) * np.float32(TWO_PI)).astype(np.float32)
    cosf = np.cos(ph).astype(np.float32)   # [T, N]; equal within (even, odd)
    sinf = np.sin(ph).astype(np.float32)
    even, odd = Q[..., 0::2], Q[..., 1::2]
    ce, se = cosf[:, 0::2], sinf[:, 0::2]  # [T, 128]
    qr_e = even * ce - odd * se            # [B, NH, T, 128]
    qr_o = odd * ce + even * se
    QRd = np.stack([qr_e, qr_o], axis=2).astype(bf_np)  # [B, NH, 2, T, 128]

    # natural layout, chunk-major so chunk-range DMA slices are contiguous:
    # [b,h][p, c, half, k] = QRd[b, h, half, c*128+p, k]
    Qn = np.ascontiguousarray(
        QRd.reshape(B, NH, 2, NCH, P, P).transpose(0, 1, 4, 3, 2, 5)
    )  # [B, NH, P, NCH, 2, P]
    # transposed layout [b,h][half, k, t] = QRd[b, h, half, t, k]
    Qt = np.ascontiguousarray(QRd.transpose(0, 1, 2, 4, 3))  # [B, NH, 2, 128, T]

    # [strict-triu | ones | strict-triu]: diag(c0) mask, cross block
    # passthrough, diag(c1) mask — matches the shared-bank G layout.
    mskT = np.ones((P, 3 * P), np.float32)
    tri = np.triu(np.ones((P, P), np.float32), k=1)
    mskT[:, :P] = tri
    mskT[:, 2 * P:] = tri
    mskT = mskT.astype(bf_np)

    V16 = V.astype(bf_np)
    # v layout [P, NCH, D]: (p, c, d) = V[c*128+p, d]
    Vp = np.ascontiguousarray(V16.reshape(B, NCH, P, D).transpose(0, 2, 1, 3))

    in_maps = []
    for core in range(NCORES):
        b = core // (NCORES // B)
        hs = (core % (NCORES // B)) * HPC
        in_maps.append(
            {
                "qn": np.ascontiguousarray(Qn[b, hs : hs + HPC]),
                "qt": np.ascontiguousarray(Qt[b, hs : hs + HPC]),
                "v": Vp[b],
                "mskT": mskT,
            }
        )
    return in_maps


def _run(inputs, trace=False, **kw):
    nc = _build()
    in_maps = _host_prep(inputs["Q_raw"], inputs["V_raw"])
    res = run_bass_kernel_spmd(nc, in_maps, list(range(NCORES)), trace=trace, **kw)
    out = np.empty((B, NH, T, D), dtype=np.float32)
    for core in range(NCORES):
        b = core // (NCORES // B)
        hs = (core % (NCORES // B)) * HPC
        # device out: [HPC, P, NCH, D] partition-major -> [HPC, T, D]
        o = res.results[core]["out"].astype(np.float32)
        out[b, hs : hs + HPC] = o.transpose(0, 2, 1, 3).reshape(HPC, T, D)
    return out, res


def kernel(**inputs):
    out, _ = _run(inputs)
    return out
